# revision 42
# baseline (speedup 1.0000x reference)
"""GAT-head message-passing kernel for 8 Trainium2 NeuronCores.

Computation (see reference):
    h  = x @ W + b                       [N, D]
    v  = leaky(h @ att_w + att_b); v = 20 - leaky(20 - v); ev = exp(v)
    num[n]  = sum_{e: row=n} a_e * (h*ev)[col_e]     [N, D]
    den[n]  = sum_{e: row=n} a_e * ev[col_e]         [N, 1]
    out = leaky(num / den)

Sharding: core c = (h, q), h = c % 2 dest-half, q = c // 2 source-quarter.
Each core computes the full feature table for its source quarter
(rows = [h*ev | ev | pad] in DRAM), gathers per-edge rows with dma_gather
(int16 indices < 25088 rows), scatter-reduces via one-hot matmuls into an
SBUF accumulator over its dest half, then a ReduceScatter(add) across the
4 cores sharing each dest half produces final sums for a distinct
quarter of dests on every core.
"""

import os

import numpy as np

# ---------------------------------------------------------------- constants
NEG_SLOPE = 0.01
CLAMP = 20.0
P = 128            # partitions / tile size
BS = 112           # dest-block width (dests per one-hot window)
# Proven-stable config: 512-idx gathers on 2 SWDGE queues, 16KB desc rings.
# The kernel is bound by Q7 SWDGE descriptor generation (~8.4ns/edge slot);
# larger batches overflow the ring (device hang), more queues don't overlap.
GBATCH = int(os.environ.get("GAT_GB", 1024))  # indices per dma_gather
TPB = GBATCH // P                             # tiles per gather batch
IDX_CHUNK = max(1, 16384 // GBATCH)            # gather batches per idx DMA
NSWQ = int(os.environ.get("GAT_NSWQ", 4))     # SWDGE queues (Q7 core pairs)
DMA_SCRATCH = int(os.environ.get("GAT_RING", 32768))
FEAT = None        # set from D at runtime: FEAT = D + 1 (65)

_prog_cache = {}


def _leaky(x):
    return np.where(x >= 0, x, NEG_SLOPE * x)


# ---------------------------------------------------------------- host prep
def _prep_core(row, col, a, h, q, NDH, NQ, NBLK):
    """Per-core edge arrays sorted by dest, blocked by 128 dests."""
    m = (row >= h * NDH) & (row < (h + 1) * NDH) & \
        (col >= q * NQ) & (col < (q + 1) * NQ)
    r = (row[m] - h * NDH).astype(np.int64)
    s = (col[m] - q * NQ).astype(np.int64)
    av = a[m].astype(np.float32)
    order = np.argsort(r, kind="stable")
    r, s, av = r[order], s[order], av[order]
    counts = np.bincount(r // BS, minlength=NBLK).astype(np.int64)
    return r, s, av, counts


def _slots_for_core(core_data, tiles_per_block):
    """Scatter a core's edges into the uniform padded slot layout."""
    r, s, av, counts = core_data
    NBLK = len(tiles_per_block)
    slots_per_block = tiles_per_block * P
    block_slot0 = np.zeros(NBLK, np.int64)
    block_slot0[1:] = np.cumsum(slots_per_block)[:-1]
    block_edge0 = np.zeros(NBLK, np.int64)
    block_edge0[1:] = np.cumsum(counts)[:-1]
    blk = r // BS
    pos = np.arange(len(r)) - block_edge0[blk]
    slot = block_slot0[blk] + pos
    T_total = int(slots_per_block.sum())
    idx = np.zeros(T_total, np.int16)
    dloc = np.zeros(T_total, np.float32)
    aval = np.zeros(T_total, np.float32)
    idx[slot] = s.astype(np.int16)
    dloc[slot] = (r % BS).astype(np.float32)
    aval[slot] = av
    return idx, dloc, aval


def _wrap_idx(idx, nbatch):
    """[T_total*P] -> [128, nbatch, GBATCH//16] wrapped + replicated."""
    w = idx.reshape(nbatch, GBATCH // 16, 16).transpose(2, 0, 1)  # [16,nb,s]
    return np.ascontiguousarray(np.tile(w, (8, 1, 1)))            # [128,nb,s]


# ---------------------------------------------------------------- program
def _build_program(N, D, NQ, NBLK, tiles_per_block, nbatch, use_bf16,
                   no_cc=False, no_gather=False):
    import concourse.bacc as bacc
    import concourse.bass as bass
    import concourse.mybir as mybir
    import concourse.tile as tile
    from concourse import library_config

    F_IN = 256
    NDH = N // 2
    TROWS = -(-NQ // P) * P          # table rows (padded quarter)
    RT = TROWS // P                  # stage-A row tiles
    FEAT = D + 1                     # 65: D feats + divide col
    TW = P                           # table width (128 cols: 512B fp32 rows)
    T_total = int(tiles_per_block.sum())
    RSROWS = NBLK * BS // 4          # rows per core after ReduceScatter
    tab_dt = mybir.dt.bfloat16 if use_bf16 else mybir.dt.float32
    xt_dt = tab_dt                   # stage-A x / W dtype matches table
    f32 = mybir.dt.float32

    # block id / first / last flags per tile
    tile_blk = np.repeat(np.arange(NBLK), tiles_per_block)
    t_first = np.zeros(T_total, bool)
    t_last = np.zeros(T_total, bool)
    ends = np.cumsum(tiles_per_block)
    t_first[ends - tiles_per_block] = True
    t_last[ends - 1] = True

    nc = bacc.Bacc("TRN2", target_bir_lowering=False, debug=False,
                   num_devices=8, num_swdge_queues=NSWQ,
                   dynamic_dma_scratch_size=DMA_SCRATCH)

    xt = nc.dram_tensor("xt", [F_IN, TROWS], xt_dt, kind="ExternalInput")
    Wsb_d = nc.dram_tensor("w_in", [F_IN, D], xt_dt, kind="ExternalInput")
    brep_d = nc.dram_tensor("b_rep", [P, D], f32, kind="ExternalInput")
    awrep_d = nc.dram_tensor("attw_rep", [P, D], f32, kind="ExternalInput")
    attb_d = nc.dram_tensor("attb_col", [P, 1], f32, kind="ExternalInput")
    cz_d = nc.dram_tensor("cz_col", [P, 2], f32, kind="ExternalInput")
    iota_d = nc.dram_tensor("iota_row", [P, P], f32, kind="ExternalInput")
    idx_d = nc.dram_tensor("idx_t", [P, nbatch, GBATCH // 16], mybir.dt.int16,
                           kind="ExternalInput")
    dloc_d = nc.dram_tensor("dloc_t", [P, T_total], f32, kind="ExternalInput")
    a_d = nc.dram_tensor("a_t", [P, T_total], f32, kind="ExternalInput")
    out_d = nc.dram_tensor("out", [BS // 4, NBLK, D], f32,
                           kind="ExternalOutput")

    with tile.TileContext(nc) as tc:
        nc.gpsimd.load_library(library_config.mlp)
        with tc.tile_pool(name="dram", bufs=1, space="DRAM") as dpool, \
             tc.tile_pool(name="persist", bufs=1) as pp:
            table = dpool.tile([TROWS, TW], tab_dt)
            acc_dram = dpool.tile([2, BS, NBLK // 2 * FEAT], tab_dt)
            rs_dram = dpool.tile([2, BS // 4, NBLK // 2 * FEAT], tab_dt)

            # persistent small tensors
            Wsb = pp.tile([P, 2, D], xt_dt)    # W as two 128-row chunks
            brep = pp.tile([P, D], f32)
            awrep = pp.tile([P, D], f32)
            attb = pp.tile([P, 1], f32)
            cz = pp.tile([P, 2], f32)
            iota = pp.tile([P, P], f32)
            iota_t = pp.tile([P, P], tab_dt)
            dloc = pp.tile([P, T_total], f32)
            aval = pp.tile([P, T_total], f32)
            vbuf = pp.tile([P, RT], f32)
            ubuf = pp.tile([P, RT], f32)
            evbuf = pp.tile([P, RT], f32)

            nc.sync.dma_start(out=Wsb[:, 0, :], in_=Wsb_d[0:P, :])
            nc.sync.dma_start(out=Wsb[:, 1, :], in_=Wsb_d[P:2 * P, :])
            nc.sync.dma_start(out=brep[:], in_=brep_d[:, :])
            nc.sync.dma_start(out=awrep[:], in_=awrep_d[:, :])
            nc.sync.dma_start(out=attb[:], in_=attb_d[:, :])
            nc.sync.dma_start(out=cz[:], in_=cz_d[:, :])
            nc.sync.dma_start(out=iota[:], in_=iota_d[:, :])
            nc.vector.tensor_copy(out=iota_t[:], in_=iota[:])
            nc.sync.dma_start(out=dloc[:], in_=dloc_d[:, :])
            nc.sync.dma_start(out=aval[:], in_=a_d[:, :])

            # ---------------- stage A: table = [(x@W+b)*ev | ev | 0] ----
            XCH = 8                    # row tiles per x chunk / table strip
            nxch = -(-RT // XCH)
            with tc.tile_pool(name="xa", bufs=2) as xa, \
                 tc.tile_pool(name="tabp", bufs=2) as tabp, \
                 tc.tile_pool(name="hbp", bufs=2) as hbp, \
                 tc.tile_pool(name="pa", bufs=2, space="PSUM") as pa:
                for ci in range(nxch):
                    t0 = ci * XCH
                    nt = min(XCH, RT - t0)
                    xch = xa.tile([P, 2, XCH * P], xt_dt, tag="xch")
                    for k in range(2):
                        nc.sync.dma_start(
                            out=xch[:, k, :nt * P],
                            in_=xt[k * P:(k + 1) * P, t0 * P:t0 * P + nt * P])
                    tabs = tabp.tile([P, XCH, TW], tab_dt, tag="tab")
                    nc.vector.memset(tabs[:, :, D + 1:], 0.0)
                    # h tiles = (x @ W); one psum bank holds the whole chunk
                    hp8 = pa.tile([P, XCH, D], f32, tag="hp8")
                    for ti in range(nt):
                        for k in range(2):
                            nc.tensor.matmul(
                                out=hp8[:, ti, :],
                                lhsT=xch[:, k, ti * P:(ti + 1) * P],
                                rhs=Wsb[:, k, :],
                                start=(k == 0), stop=(k == 1))
                    # hb = h + b; v = sum_f hb * att_w  (chunk-batched DVE)
                    hb8 = hbp.tile([P, XCH, D], f32, tag="hb8")
                    nc.vector.tensor_tensor(
                        out=hb8[:, :nt, :], in0=hp8[:, :nt, :],
                        in1=brep[:, None, :].to_broadcast([P, nt, D]),
                        op=mybir.AluOpType.add)
                    nc.vector.tensor_copy(
                        out=tabs[:, :nt, 0:D], in_=hb8[:, :nt, :])
                    scr8 = hbp.tile([P, XCH, D], f32, tag="scr8")
                    nc.vector.tensor_tensor(
                        out=scr8[:, :nt, :], in0=hb8[:, :nt, :],
                        in1=awrep[:, None, :].to_broadcast([P, nt, D]),
                        op=mybir.AluOpType.mult)
                    nc.vector.tensor_reduce(
                        out=vbuf[:, t0:t0 + nt, None], in_=scr8[:, :nt, :],
                        axis=mybir.AxisListType.X, op=mybir.AluOpType.add)
                    # ev = exp(20 - leaky(20 - leaky(v + att_b)))
                    # leaky(x) = max(x, 0.01 x) composed on DVE
                    u = ubuf[:, t0:t0 + nt]
                    w = vbuf[:, t0:t0 + nt]
                    nc.vector.tensor_scalar(
                        u, w, attb[:], None, mybir.AluOpType.add)
                    nc.vector.scalar_tensor_tensor(
                        out=u, in0=u, scalar=NEG_SLOPE, in1=u,
                        op0=mybir.AluOpType.mult, op1=mybir.AluOpType.max)
                    nc.vector.tensor_scalar(
                        u, u, -1.0, CLAMP,
                        mybir.AluOpType.mult, mybir.AluOpType.add)
                    nc.vector.scalar_tensor_tensor(
                        out=u, in0=u, scalar=NEG_SLOPE, in1=u,
                        op0=mybir.AluOpType.mult, op1=mybir.AluOpType.max)
                    nc.scalar.activation(
                        out=evbuf[:, t0:t0 + nt], in_=u,
                        func=mybir.ActivationFunctionType.Exp,
                        bias=cz[:, 0:1], scale=-1.0, alpha=0.0)
                    # tabs[:, :, 0:D] *= ev ; tabs[:, :, D] = ev  (batched)
                    nc.vector.tensor_tensor(
                        out=tabs[:, :nt, 0:D], in0=tabs[:, :nt, 0:D],
                        in1=evbuf[:, t0:t0 + nt, None]
                            .to_broadcast([P, nt, D]),
                        op=mybir.AluOpType.mult)
                    nc.vector.tensor_copy(
                        out=tabs[:, :nt, D:D + 1],
                        in_=evbuf[:, t0:t0 + nt, None])
                    # row r of quarter stored at table[(r % P) * RT + r // P]
                    nc.sync.dma_start(
                        out=table[:, :].rearrange("(p t) w -> p t w", p=P)
                            [:, t0:t0 + nt, :],
                        in_=tabs[:, :nt, :])

            # ---------------- stage B: gather + one-hot matmul reduce ---
            HB = NBLK // 2              # blocks per accumulator half
            # batch index after which every tile of half 0 has been reduced
            half_tiles = int(tiles_per_block[:HB].sum())
            bi_half = (half_tiles + TPB - 1) // TPB - 1

            BQ = BS // 4
            NCH = 8
            JC = NBLK // NCH

            def _rs_and_finale(h, acch, fin):
                """Spill acc half h, ReduceScatter it, apply num/den+leaky."""
                rsv = rs_dram[h].rearrange("p (j f) -> p j f", f=FEAT)
                nc.sync.dma_start(out=acc_dram[h], in_=acch[:BS, :, :])
                if no_cc:
                    nc.sync.dma_start(out=rs_dram[h],
                                      in_=acc_dram[h][0:BS // 4, :])
                else:
                    nc.gpsimd.collective_compute(
                        "ReduceScatter",
                        mybir.AluOpType.add,
                        replica_groups=[[0, 2, 4, 6], [1, 3, 5, 7]],
                        ins=[acc_dram[h][:, :].opt()],
                        outs=[rs_dram[h][:, :].opt()],
                    )
                for cj in range(NCH // 2):
                    js = slice(cj * JC, (cj + 1) * JC)
                    osl = slice(h * HB + cj * JC, h * HB + (cj + 1) * JC)
                    racc = fin.tile([BQ, JC, FEAT], tab_dt, tag="racc")
                    nc.sync.dma_start(out=racc[:], in_=rsv[:, js, :])
                    raf = fin.tile([BQ, JC, FEAT], f32, tag="raf")
                    nc.vector.tensor_copy(out=raf[:], in_=racc[:])
                    recip = fin.tile([BQ, JC], f32, tag="recip")
                    # clamp: zero-degree / pad dests give 0 instead of inf
                    nc.vector.tensor_scalar(
                        recip[:], raf[:, :, D], 1e-30, None,
                        mybir.AluOpType.max)
                    nc.vector.reciprocal(out=recip[:], in_=recip[:])
                    osb = fin.tile([BQ, JC, D], f32, tag="osb")
                    nc.vector.scalar_tensor_tensor(
                        out=osb[:], in0=raf[:, :, 0:D], scalar=1.0,
                        in1=recip[:, :, None].to_broadcast([BQ, JC, D]),
                        op0=mybir.AluOpType.mult, op1=mybir.AluOpType.mult)
                    nc.vector.scalar_tensor_tensor(
                        out=osb[:], in0=osb[:], scalar=NEG_SLOPE, in1=osb[:],
                        op0=mybir.AluOpType.mult, op1=mybir.AluOpType.max)
                    nc.sync.dma_start(out=out_d[:, osl, :], in_=osb[:])

            with tc.tile_pool(name="accp", bufs=1) as accp, \
                 tc.tile_pool(name="idxp", bufs=3) as idxp, \
                 tc.tile_pool(name="msgp", bufs=6) as msgp, \
                 tc.tile_pool(name="sp", bufs=3) as sp, \
                 tc.tile_pool(name="pb", bufs=4, space="PSUM") as pb, \
                 tc.tile_pool(name="finc", bufs=1) as finc:
                acc = [accp.tile([P, HB, FEAT], tab_dt, tag=f"acc{h}",
                                 name=f"acc{h}")
                       for h in range(2)]
                psum_cur = None
                for bi in range(nbatch):
                    if bi % IDX_CHUNK == 0:
                        nb = min(IDX_CHUNK, nbatch - bi)
                        idxs = idxp.tile([P, IDX_CHUNK, GBATCH // 16],
                                         mybir.dt.int16, tag="idx")
                        nc.sync.dma_start(
                            out=idxs[:, :nb, :],
                            in_=idx_d[:, bi:bi + nb, :])
                    msgs = msgp.tile([P, TPB, TW], tab_dt, tag="msg")
                    if no_gather:
                        for _tt in range(TPB):
                            nc.sync.dma_start(
                                out=msgs[:, _tt, :],
                                in_=table[0:P, :])
                    else:
                        nc.gpsimd.dma_gather(
                            out_ap=msgs[:],
                            in_ap=table[:, :],
                            idxs_ap=idxs[:, bi % IDX_CHUNK, :],
                            num_idxs=GBATCH,
                            num_idxs_reg=GBATCH,
                            elem_size=TW,
                            elem_step=TW,
                            single_packet=os.environ.get("GAT_SP", "1") == "1",
                            queue_num=bi % NSWQ,
                        )
                    nt = min(TPB, T_total - bi * TPB)
                    t0b = bi * TPB
                    # batch-build TPB pure one-hots in one DVE op; a_e is
                    # folded into msgs below instead of into S
                    S8 = sp.tile([P, TPB, BS], tab_dt, tag="S8")
                    nc.vector.scalar_tensor_tensor(
                        out=S8[:, :nt, :],
                        in0=iota_t[:, None, :BS].to_broadcast([P, nt, BS]),
                        scalar=1.0,
                        in1=dloc[:, t0b:t0b + nt, None]
                            .to_broadcast([P, nt, BS]),
                        op0=mybir.AluOpType.mult,
                        op1=mybir.AluOpType.is_equal)
                    nc.vector.scalar_tensor_tensor(
                        out=msgs[:, :nt, 0:FEAT],
                        in0=msgs[:, :nt, 0:FEAT],
                        scalar=1.0,
                        in1=aval[:, t0b:t0b + nt, None]
                            .to_broadcast([P, nt, FEAT]),
                        op0=mybir.AluOpType.mult,
                        op1=mybir.AluOpType.mult)
                    for tt in range(nt):
                        t = t0b + tt
                        j = int(tile_blk[t])
                        if t_first[t]:
                            psum_cur = pb.tile([BS, FEAT], f32, tag="pblk")
                        nc.tensor.matmul(
                            out=psum_cur[:],
                            lhsT=S8[:, tt, :],
                            rhs=msgs[:, tt, 0:FEAT],
                            start=bool(t_first[t]), stop=bool(t_last[t]))
                        if t_last[t]:
                            nc.any.tensor_copy(
                                out=acc[j // HB][:BS, j % HB, :],
                                in_=psum_cur[:])
                    if bi == bi_half:
                        _rs_and_finale(0, acc[0], finc)
                _rs_and_finale(1, acc[1], finc)
    nc.finalize()
    return nc


def _install_ntff_hook(bass_utils):
    """Dev-only: register the axon NTFF profile hook + skip artifact upload."""
    import sys
    import types
    bass_utils.upload_artifacts = lambda tmpdir: "local://" + tmpdir
    try:
        from antenv.axon_hooks import get_axon_ntff_profile_hook  # noqa: F401
        return
    except ImportError:
        pass
    mod = types.ModuleType("antenv.axon_hooks")
    mod._hook = None
    mod.set_axon_ntff_profile_hook = lambda h: setattr(mod, "_hook", h)
    mod.get_axon_ntff_profile_hook = lambda: mod._hook
    sys.modules["antenv.axon_hooks"] = mod
    if "/root/.axon_site" not in sys.path:
        sys.path.insert(0, "/root/.axon_site")
    from trn_agent_boot.trn_boot import _ntff_profile_via_ctypes
    h = _ntff_profile_via_ctypes("/opt/axon/libaxon_pjrt.so")
    if h is not None:
        mod._hook = h


# ---------------------------------------------------------------- entry
def kernel(x, edge_index, adj_values, W, b, att_w, att_b):
    x = np.asarray(x, np.float32)
    edge_index = np.asarray(edge_index)
    adj_values = np.asarray(adj_values, np.float32)
    W = np.asarray(W, np.float32)
    b = np.asarray(b, np.float32)
    att_w = np.asarray(att_w, np.float32)
    att_b = np.asarray(att_b, np.float32)

    N, F_IN = x.shape
    D = W.shape[1]
    NDH, NQ = N // 2, N // 4
    # NBLK * BS must be divisible by 512 so ReduceScatter rows split into
    # whole 128-partition tiles per core: BS=112 -> NBLK multiple of 32
    NBLK = max(32, -(-(-(-NDH // BS)) // 32) * 32)
    TROWS = -(-NQ // P) * P
    use_bf16 = os.environ.get("GAT_BF16", "1") == "1"
    no_cc = os.environ.get("GAT_NOCC", "0") == "1"
    no_gather = os.environ.get("GAT_NOGATHER", "0") == "1"

    row = np.asarray(edge_index[0])
    col = np.asarray(edge_index[1])

    cores = list(range(8))
    data = [_prep_core(row, col, adj_values, c % 2, c // 2, NDH, NQ, NBLK)
            for c in cores]
    tiles_per_block = np.maximum(
        1, -(-np.stack([d[3] for d in data]) // P)).max(axis=0)
    # pad T_total to a multiple of TPB using the last (fake-dest) block
    T_total = int(tiles_per_block.sum())
    tiles_per_block[-1] += (-T_total) % TPB
    T_total = int(tiles_per_block.sum())
    nbatch = T_total // TPB

    key = (N, D, NQ, NBLK, nbatch, use_bf16, no_cc, no_gather,
           GBATCH, NSWQ, DMA_SCRATCH,
           tuple(tiles_per_block.tolist()))
    if key not in _prog_cache:
        _prog_cache[key] = _build_program(
            N, D, NQ, NBLK, tiles_per_block, nbatch, use_bf16,
            no_cc=no_cc, no_gather=no_gather)
    nc = _prog_cache[key]

    brep = np.ascontiguousarray(np.broadcast_to(b, (P, D)), dtype=np.float32)
    awrep = np.ascontiguousarray(
        np.broadcast_to(att_w[:, 0], (P, D)), dtype=np.float32)
    attb_col = np.full((P, 1), float(att_b[0]), np.float32)
    cz_col = np.zeros((P, 2), np.float32)
    cz_col[:, 0] = CLAMP
    iota_row = np.ascontiguousarray(
        np.broadcast_to(np.arange(P, dtype=np.float32), (P, P)))

    if use_bf16:
        import ml_dtypes
        xt_np = ml_dtypes.bfloat16
    else:
        xt_np = np.float32
    W_in = np.ascontiguousarray(W.astype(xt_np))

    in_maps = []
    for c in cores:
        q = c // 2
        xs = np.zeros((F_IN, TROWS), xt_np)
        xs[:, :NQ] = x[q * NQ:(q + 1) * NQ].T.astype(xt_np)
        idx, dloc, aval = _slots_for_core(data[c], tiles_per_block)
        # table rows are stored permuted: row r lives at (r % P) * RT + r // P
        RT = TROWS // P
        idx = ((idx % P) * RT + idx // P).astype(np.int16)
        in_maps.append({
            "xt": xs,
            "w_in": W_in,
            "b_rep": brep,
            "attw_rep": awrep,
            "attb_col": attb_col,
            "cz_col": cz_col,
            "iota_row": iota_row,
            "idx_t": _wrap_idx(idx, nbatch),
            "dloc_t": np.ascontiguousarray(dloc.reshape(-1, P).T),
            "a_t": np.ascontiguousarray(aval.reshape(-1, P).T),
        })

    if os.environ.get("GAT_SIM", "0") == "1":
        from concourse.bass_interp import MultiCoreSim
        sim = MultiCoreSim(nc, 8)
        for c in cores:
            for k, v in in_maps[c].items():
                sim.cores[c].tensor(k)[:] = v
        sim.simulate()

        class _R:
            results = [{"out": np.array(sim.cores[c].tensor("out"))}
                       for c in cores]
        res = _R()
    else:
        import concourse.bass_utils as bass_utils
        from concourse.bass_utils import run_bass_kernel_spmd
        trace = os.environ.get("GAT_TRACE", "0") == "1"
        if trace:
            _install_ntff_hook(bass_utils)
        res = run_bass_kernel_spmd(nc, in_maps, cores, trace=trace)
        if trace and res.exec_time_ns is not None:
            print(f"HW exec time: {res.exec_time_ns} ns")
            print(f"mean exec time: {res.mean_exec_time_ns} ns")

    out = np.empty((N, D), np.float32)
    BQ = BS // 4
    j_grid = np.arange(NBLK)
    for c in cores:
        h, q = c % 2, c // 2
        o = res.results[c]["out"]            # [BQ, NBLK, D]
        for p in range(BQ):
            d = j_grid * BS + (q * BQ + p)   # dests for this partition row
            m = d < NDH
            out[h * NDH + d[m]] = o[p][m]
    return out

